# revision 35
# baseline (speedup 1.0000x reference)
"""Trainium2 Bass kernel for a pre-LN causal decoder layer (MHA + SwiGLU).

Sharding: 2-way data parallel over batch x 4-way tensor parallel over heads.
Core c (of 8): batch b=c//4, group rank r=c%4, heads [4r, 4r+4).
Each core computes Q/K/V + causal attention for its 4 heads over its batch's
2048 tokens, a partial ctx @ Wo[rows], then per-J-block ReduceScatter(add)
over the 4-core group hands each core complete attention output for
4 x 128 tokens (token J*512 + r*128). The FFN (SwiGLU, full weights) then
runs token-parallel on those 512 tokens; the host concatenates the shards.

v2 notes:
 - QKV and Wo run in fp8 DoubleRow (x32 scaling), scores/AV stay bf16.
 - W3 is a single fp8 DR matmul (no error-feedback terms).
 - LayerNorm folded: stats from fp8 x^T via ones-matmuls, -mu rank-1 fixup
   in bf16, rstd applied to PSUM (with 1/1024 descale folded in).
 - One ReduceScatter per 512-token J-block on dedicated DRAM tensors.
 - causal mask: gpsimd affine_select zeroing exp() output (diag tiles).
 - reciprocal_approx_fast instead of vector.reciprocal.
 - softmax denominators use a 1/64 ones-row so 1/denom broadcast also
   applies the x64 fp8 ctx scale for free.
 - FFN weights (W1/W2/W3 fp8) prefetched into SBUF during attention;
   single 512-token FFN pass, W3 accumulated per 128-token tile.
"""

import sys

sys.path.insert(0, "/opt/trn_rl_repo")

import numpy as np
import ml_dtypes

import concourse.bass as bass
import concourse.mybir as mybir
import concourse.tile as tile
from concourse import bacc
from concourse.bass_utils import run_bass_kernel_spmd
from concourse.masks import make_identity



BF16 = ml_dtypes.bfloat16
FP8 = ml_dtypes.float8_e4m3
F32 = mybir.dt.float32
BF = mybir.dt.bfloat16
F8 = mybir.dt.float8e4
I16 = mybir.dt.int16
DR = mybir.MatmulPerfMode.DoubleRow
# Schraudolph exp in bf16-bit domain: bf16_bits(e^x) ~= x*128*log2(e) + bias
EXP_K = 184.66496
EXP_B = 16249.0

B, T, C = 2, 2048, 1024
H, HS = 16, 64
HID = 2730
HIDP = 2816  # padded to 22*128
NF = HIDP // 128  # 22
HPC = 4  # heads per core
TLOC = T // 4  # 512 tokens owned post-RS
EPS = 1e-3
RG = [[0, 1, 2, 3], [4, 5, 6, 7]]
NT = T // 128  # 16 token tiles
NJ = T // 512  # 4 t-blocks of 512
NKC = C // 128  # 8 contraction chunks
S = 32.0  # fp8 scale
S2 = S * S  # 1024

_cache = {}
DBG = None  # debug tap: 'qkv' dumps qT2, 'ctx' dumps ctxT, 'hn2' dumps hn2T


def _build(have_bw, sim=False):
    nc = bacc.Bacc(None, target_bir_lowering=False, debug=False)
    # all host-side tensors are partition-major for contiguous DMA
    xT8 = nc.declare_dram_parameter("xT8", [128, NKC * T], F8, isOutput=False)
    xres = nc.declare_dram_parameter("xres", [128, 4 * C], BF, isOutput=False)
    wqkv = nc.declare_dram_parameter("wqkv", [128, NKC * 768], F8, isOutput=False)
    gws = nc.declare_dram_parameter("gws", [768], BF, isOutput=False)
    if have_bw:
        bw = nc.declare_dram_parameter("bw", [768], F32, isOutput=False)
        bw1 = nc.declare_dram_parameter("bw1", [HIDP], F32, isOutput=False)
        bw2 = nc.declare_dram_parameter("bw2", [HIDP], F32, isOutput=False)
    wo = nc.declare_dram_parameter("wo", [128, 2 * C], F8, isOutput=False)
    w12 = nc.declare_dram_parameter("w12", [128, NF * 16 * 128], F8, isOutput=False)
    w3 = nc.declare_dram_parameter("w3", [128, NF * C], F8, isOutput=False)
    out = nc.declare_dram_parameter("out", [128, 4 * C], BF, isOutput=True)
    dbg = (nc.declare_dram_parameter("dbg", [128, 2 * T], BF, isOutput=True)
           if DBG else None)

    rs_in = [nc.dram_tensor(f"rs_in{j}", [512, C], BF) for j in range(NJ)]
    rs_out = [nc.dram_tensor(f"rs_out{j}", [128, C], BF) for j in range(NJ)]

    with tile.TileContext(nc) as tc:
        from contextlib import ExitStack
        with ExitStack() as ctx:
            consts = ctx.enter_context(tc.tile_pool(name="consts", bufs=1))
            ident = consts.tile([128, 128], BF)
            make_identity(nc, ident)
            epsc = consts.tile([128, 1], F32)
            # Sqrt bias for phase-A stats: 1024^2 * eps (vhat = 1024*var)
            nc.vector.memset(epsc, S2 * S2 * EPS)
            epsc2 = consts.tile([128, 1], F32)
            # Sqrt bias for FFN LN (sd/32 trick): eps/1024
            nc.vector.memset(epsc2, EPS / S2)
            gws_row = consts.tile([1, 768], BF)
            nc.sync.dma_start(out=gws_row, in_=gws[:].rearrange("(o j) -> o j", o=1))
            if have_bw:
                bw_cols = consts.tile([128, 6], F32)
                nc.sync.dma_start(out=bw_cols, in_=bw[:].rearrange("(j p) -> p j", p=128))
                bw1c = consts.tile([128, NF], F32)
                nc.sync.dma_start(out=bw1c, in_=bw1[:].rearrange("(f p) -> p f", p=128))
                bw2c = consts.tile([128, NF], F32)
                nc.sync.dma_start(out=bw2c, in_=bw2[:].rearrange("(f p) -> p f", p=128))
            wqkv_sb = consts.tile([128, NKC, 768], F8)
            nc.sync.dma_start(out=wqkv_sb,
                              in_=wqkv[:].rearrange("p (k j) -> p k j", k=NKC))
            wo_sb = consts.tile([128, 2, C], F8)
            nc.sync.dma_start(out=wo_sb,
                              in_=wo[:].rearrange("p (k j) -> p k j", k=2))

            # persistent activation tiles
            pA = ctx.enter_context(tc.tile_pool(name="pA", bufs=1))
            qT2 = pA.tile([128, 2, T], BF)   # [256 q-dims, T]
            kT2 = pA.tile([128, 2, T], BF)
            v_sb = pA.tile([128, NT, HPC, 65], BF)
            # ones row at 1/32: denominators come out as denom/32, so the
            # reciprocal directly yields the 32/denom needed for fp8 ctx.
            nc.vector.memset(v_sb[:, :, :, 64:65], 1.0 / 32.0)

            pE = ctx.enter_context(tc.tile_pool(name="pE", bufs=1))
            ctxT = pE.tile([128, 2, T], F8)   # [256 head-dims, T] x64 scale
            out1 = pE.tile([128, 4, C], BF)
            hn2T = pE.tile([128, NKC, TLOC], F8)   # 32*LN(x1) transposed
            g_sb = pE.tile([128, NF, TLOC], F8)    # 32*g
            w12_sb = pE.tile([128, NF, 16, 128], F8)
            w3_sb = pE.tile([128, NF, C], F8)
            xr_sb = pE.tile([128, 4, C], BF)

            # ------------- Phase A: row-stats + fused-LN QKV (qkv^T) -----
            with tc.tile_pool(name="xTp", bufs=1) as xTp, \
                 tc.tile_pool(name="vTp", bufs=1) as vTp, \
                 tc.tile_pool(name="rowp", bufs=1) as rowp, \
                 tc.tile_pool(name="xsqp", bufs=2) as xsqp, \
                 tc.tile_pool(name="strow", bufs=2) as strow, \
                 tc.tile_pool(name="psS", bufs=1, space="PSUM") as psS, \
                 tc.tile_pool(name="psA", bufs=3, space="PSUM") as psA, \
                 tc.tile_pool(name="psV", bufs=2, space="PSUM") as psV:
                xT_sb = xTp.tile([128, NKC, T], F8)
                xTr = xT8[:].rearrange("p (k t) -> p k t", k=NKC)
                for kc in range(NKC):
                    nc.sync.dma_start(out=xT_sb[:, kc, :], in_=xTr[:, kc, :])
                # FFN weights + residual prefetch (DMA idle during attention)
                nc.sync.dma_start(out=xr_sb,
                                  in_=xres[:].rearrange("p (a c) -> p a c", a=4))
                nc.sync.dma_start(
                    out=w12_sb, in_=w12[:].rearrange(
                        "p (f g j) -> p f g j", f=NF, g=16))
                nc.sync.dma_start(
                    out=w3_sb, in_=w3[:].rearrange("p (f j) -> p f j", f=NF))

                vT_tmp = vTp.tile([128, 2, T], BF)
                # DR ldweights needs the Ko-pair step to be 16B-aligned
                onesc_t = rowp.tile([128, 2, 16], F8)
                nc.vector.memset(onesc_t, 1.0)
                onesc = onesc_t[:, :, 0:1]

                dests = [(qT2, 0), (qT2, 1), (kT2, 0), (kT2, 1),
                         (vT_tmp, 0), (vT_tmp, 1)]
                # per 512-token block: stats then fused-LN QKV
                for tq in range(4):
                    sl = slice(tq * 512, (tq + 1) * 512)
                    # token stats via DR ones-matmuls on x8 and x8^2/64
                    # x8^2/256 keeps the e4m3 cast below its 240 max
                    xsq_sb = xsqp.tile([128, NKC, 512], F8, tag="xsq")
                    for kc in range(NKC):
                        nc.vector.scalar_tensor_tensor(
                            out=xsq_sb[:, kc, :], in0=xT_sb[:, kc, sl],
                            scalar=1.0 / 256.0, in1=xT_sb[:, kc, sl],
                            op0=mybir.AluOpType.mult, op1=mybir.AluOpType.mult)
                    mu_ps = psS.tile([1, 512], F32, tag="mu")
                    sq_ps = psS.tile([1, 512], F32, tag="sq")
                    for k2 in range(NKC // 2):
                        nc.tensor.matmul(mu_ps, onesc,
                                         xT_sb[:, 2 * k2:2 * k2 + 2, sl],
                                         start=(k2 == 0), stop=(k2 == 3),
                                         perf_mode=DR)
                        nc.tensor.matmul(sq_ps, onesc,
                                         xsq_sb[:, 2 * k2:2 * k2 + 2, :],
                                         start=(k2 == 0), stop=(k2 == 3),
                                         perf_mode=DR)
                    # muhat = 32*mu ; vhat = 1024*var = 256*sq/C - muhat^2
                    negmu_bf = strow.tile([1, 512], BF, tag="nmu")
                    nc.vector.tensor_scalar_mul(negmu_bf, mu_ps, -1.0 / C)
                    mu2 = strow.tile([1, 512], F32, tag="mu2")
                    nc.vector.tensor_mul(mu2, negmu_bf, negmu_bf)
                    var = strow.tile([1, 512], F32, tag="var")
                    nc.vector.scalar_tensor_tensor(
                        out=var, in0=sq_ps, scalar=256.0 / C, in1=mu2,
                        op0=mybir.AluOpType.mult, op1=mybir.AluOpType.subtract)
                    # sd1024 = sqrt(1024*vhat + 1024^2 eps) = 1024*sd
                    sd = strow.tile([1, 512], F32, tag="sd")
                    nc.scalar.activation(out=sd, in_=var,
                                         func=mybir.ActivationFunctionType.Sqrt,
                                         scale=float(S2), bias=epsc[0:1, 0:1])
                    rstd_row = strow.tile([1, 512], F32, tag="rstd")
                    nc.vector.reciprocal_approx_fast(rstd_row, sd)
                    rstd_b = strow.tile([128, 512], F32, tag="rstdb")
                    nc.gpsimd.partition_broadcast(rstd_b, rstd_row)

                    # qkv^T = W^T x^T + (-muhat) x gws, then x rstd/1024
                    for jb in range(6):
                        dst, dslot = dests[jb]
                        jsl = slice(jb * 128, (jb + 1) * 128)
                        ps = psA.tile([128, 512], F32, tag="qkv")
                        for k2 in range(NKC // 2):
                            nc.tensor.matmul(ps,
                                             wqkv_sb[:, 2 * k2:2 * k2 + 2, jsl],
                                             xT_sb[:, 2 * k2:2 * k2 + 2, sl],
                                             start=(k2 == 0), stop=False,
                                             perf_mode=DR)
                        nc.tensor.matmul(ps, gws_row[0:1, jsl],
                                         negmu_bf[0:1, :],
                                         start=False, stop=True)
                        if have_bw:
                            tmpf = strow.tile([128, 512], F32, tag="tmpf")
                            nc.vector.tensor_mul(tmpf, ps, rstd_b)
                            nc.vector.tensor_scalar_add(dst[:, dslot, sl], tmpf,
                                                        bw_cols[:, jb:jb + 1])
                        else:
                            nc.vector.tensor_mul(dst[:, dslot, sl], ps,
                                                 rstd_b)

                # transpose v back to [s, d] per head
                for h in range(HPC):
                    po = (h % 2) * 64
                    idn = ident[po:po + 64, po:po + 64]
                    for si in range(NT):
                        tp = psV.tile([128, 64], BF, tag="tpv")
                        nc.tensor.transpose(
                            tp, vT_tmp[po:po + 64, h // 2,
                                       si * 128:(si + 1) * 128], idn)
                        nc.vector.tensor_copy(v_sb[:, si, h, 0:64], tp)

            if DBG == 'qkv':
                nc.sync.dma_start(
                    out=dbg[:].rearrange("p (a t) -> p a t", a=2), in_=qT2)
            if DBG == 'k':
                nc.sync.dma_start(
                    out=dbg[:].rearrange("p (a t) -> p a t", a=2), in_=kT2)

            # ---- Phase B: attention (J-outer) + Wo (fp8 DR) + per-J RS ----
            attnp = ctx.enter_context(tc.tile_pool(name="attnp", bufs=3))
            with tc.tile_pool(name="scps", bufs=4, space="PSUM") as scps, \
                 tc.tile_pool(name="avps", bufs=2, space="PSUM") as avps, \
                 tc.tile_pool(name="wops", bufs=1, space="PSUM") as wops, \
                 tc.tile_pool(name="band", bufs=6) as bandp, \
                 tc.tile_pool(name="stC", bufs=3) as stC:

                for J in range(NJ):
                    for h in range(HPC):
                        po = (h % 2) * 64
                        hs = h // 2
                        av = avps.tile([65, 512], F32, tag="av")
                        nst = 4 * J + 4  # s-tiles 0..nst-1

                        def emit_sc(i):
                            w = i - 4 * J
                            off = max(w, 0) * 128
                            sc = scps.tile([128, 512], F32, tag="sc")
                            nc.tensor.matmul(
                                sc[:, off:512],
                                kT2[po:po + 64, hs, i * 128:(i + 1) * 128],
                                qT2[po:po + 64, hs, J * 512 + off:(J + 1) * 512],
                                start=True, stop=True)
                            if w < 0:
                                # full (all-causal) tile: Schraudolph exp on
                                # the vector engine -- int16 bits of bf16 e^x
                                bdi = bandp.tile([128, 512], I16, tag="bdi")
                                nc.vector.tensor_scalar(
                                    bdi, sc, EXP_K, EXP_B,
                                    mybir.AluOpType.mult, mybir.AluOpType.add)
                                return bdi.bitcast(BF), 0
                            bd = bandp.tile([128, 512], BF, tag="bd")
                            nc.scalar.activation(out=bd[:, off:512],
                                                 in_=sc[:, off:512],
                                                 func=mybir.ActivationFunctionType.Exp)
                            # zero strictly-upper entries of diag tile
                            nc.gpsimd.affine_select(
                                out=bd[:, off:off + 128],
                                in_=bd[:, off:off + 128],
                                compare_op=mybir.AluOpType.is_ge,
                                fill=0.0, base=0, pattern=[[1, 128]],
                                channel_multiplier=-1)
                            return bd, off

                        def emit_av(i, bd_off):
                            bd, off = bd_off
                            nc.tensor.matmul(
                                av[:, off:512],
                                v_sb[:, i, h, :],
                                bd[:, off:512],
                                start=(i == 0), stop=(i == nst - 1))

                        # software pipeline with 2 score tiles in flight so
                        # the exp latency hides under PE work
                        from collections import deque
                        pend = deque()
                        for i in range(nst):
                            pend.append(emit_sc(i))
                            if len(pend) > 2:
                                emit_av(i - 2, pend.popleft())
                        for k, bd_off in enumerate(pend):
                            emit_av(nst - len(pend) + k, bd_off)
                        # normalize: rows 0..63 are ctx^T, row 64 = denom/32.
                        den = stC.tile([1, 512], F32, tag="den")
                        nc.scalar.copy(den, av[64:65, :])
                        rrow = stC.tile([1, 512], F32, tag="rr")
                        nc.vector.reciprocal_approx_fast(rrow, den)
                        rb64 = stC.tile([64, 512], F32, tag="rb")
                        nc.gpsimd.partition_broadcast(rb64, rrow)
                        nc.vector.tensor_mul(
                            ctxT[po:po + 64, hs, J * 512:(J + 1) * 512],
                            av[0:64, :], rb64)
                    # Wo (fp8 DR over the 256-dim contraction) per token tile
                    for tl in range(4):
                        ti = J * 4 + tl
                        wp = wops.tile([128, C], F32, tag="wp")
                        for n0 in (0, 512):
                            nc.tensor.matmul(
                                wp[:, n0:n0 + 512],
                                ctxT[:, :, ti * 128:(ti + 1) * 128],
                                wo_sb[:, :, n0:n0 + 512],
                                start=True, stop=True, perf_mode=DR)
                        at = attnp.tile([128, C], BF, tag="at")
                        # wp is 1024x true attn (32*32); descale + cast
                        nc.scalar.mul(at, wp, 1.0 / 1024.0)
                        nc.sync.dma_start(
                            out=rs_in[J][tl * 128:(tl + 1) * 128, :], in_=at)
                    nc.gpsimd.collective_compute(
                        "ReduceScatter", mybir.AluOpType.add,
                        replica_groups=RG,
                        ins=[rs_in[J][:, :]],
                        outs=[rs_out[J][:, :]])

            # ------- Phase C: post-RS LN prep (per 128-token chunk) ------
            with tc.tile_pool(name="stE", bufs=2) as stE, \
                 tc.tile_pool(name="rsp", bufs=2) as rsp, \
                 tc.tile_pool(name="psE", bufs=2, space="PSUM") as psE, \
                 tc.tile_pool(name="gtmp", bufs=3) as gtmp, \
                 tc.tile_pool(name="psG", bufs=2, space="PSUM") as psG:
                for ck in range(4):
                    rs_t = rsp.tile([128, C], BF, tag="rs")
                    nc.sync.dma_start(out=rs_t, in_=rs_out[ck][:, :])
                    o1t = out1[:, ck, :]
                    nc.vector.tensor_add(o1t, xr_sb[:, ck, :], rs_t)
                    st = stE.tile([128, 2, 6], F32, tag="st")
                    nc.vector.bn_stats(out=st[:, 0, :], in_=o1t[:, 0:512])
                    nc.vector.bn_stats(out=st[:, 1, :], in_=o1t[:, 512:1024])
                    mv = stE.tile([128, 2], F32, tag="mv")
                    nc.vector.bn_aggr(out=mv, in_=st)
                    # sd/32 = sqrt((var+eps)/1024); recip -> 32*rstd
                    sd = stE.tile([128, 1], F32, tag="sd")
                    nc.scalar.activation(out=sd, in_=mv[:, 1:2],
                                         func=mybir.ActivationFunctionType.Sqrt,
                                         scale=float(1.0 / S2),
                                         bias=epsc2[:, 0:1])
                    rstd = stE.tile([128, 1], F32, tag="rstd")
                    nc.vector.reciprocal_approx_fast(rstd, sd)
                    rmu = stE.tile([128, 1], F32, tag="rmu")
                    nc.vector.tensor_mul(rmu, mv[:, 0:1], rstd)
                    hn2 = stE.tile([128, C], BF, tag="hn2")
                    # hn2 = 32*LN(x1)
                    nc.vector.tensor_scalar(hn2, o1t, rstd, rmu,
                                            mybir.AluOpType.mult,
                                            mybir.AluOpType.subtract)
                    for kc in range(NKC):
                        tp = psE.tile([128, 128], BF, tag="tpE")
                        nc.tensor.transpose(tp, hn2[:, kc * 128:(kc + 1) * 128],
                                            ident)
                        nc.vector.tensor_copy(
                            hn2T[:, kc, ck * 128:(ck + 1) * 128], tp)

                # ---- Phase D: SwiGLU FFN, single 512-token pass ----
                for fi in range(NF):
                    gps = psG.tile([128, 2, 512], F32, tag="g12")
                    g1 = gps[:, 0, :]
                    g2 = gps[:, 1, :]
                    for k2 in range(4):
                        nc.tensor.matmul(g1, w12_sb[:, fi, 2 * k2:2 * k2 + 2, :],
                                         hn2T[:, 2 * k2:2 * k2 + 2, :],
                                         start=(k2 == 0), stop=(k2 == 3),
                                         perf_mode=DR)
                    # g1 = 1024 * preact
                    sil = gtmp.tile([128, 512], BF, tag="sil")
                    if sim:
                        sg = gtmp.tile([128, 512], BF, tag="sg")
                        nc.scalar.activation(out=sg, in_=g1,
                                             func=mybir.ActivationFunctionType.Sigmoid,
                                             scale=1.0 / S2,
                                             bias=(bw1c[:, fi:fi + 1] if have_bw else 0.0))
                        tsil = gtmp.tile([128, 512], F32, tag="tsil")
                        nc.gpsimd.tensor_scalar_mul(tsil, g1, 1.0 / S2)
                        nc.gpsimd.tensor_mul(sil, tsil, sg)
                    else:
                        nc.scalar.activation(out=sil, in_=g1,
                                             func=mybir.ActivationFunctionType.Silu,
                                             scale=1.0 / S2,
                                             bias=(bw1c[:, fi:fi + 1] if have_bw else 0.0))
                    for k2 in range(4):
                        nc.tensor.matmul(g2, w12_sb[:, fi, 8 + 2 * k2:10 + 2 * k2, :],
                                         hn2T[:, 2 * k2:2 * k2 + 2, :],
                                         start=(k2 == 0), stop=(k2 == 3),
                                         perf_mode=DR)
                    if have_bw:
                        nc.vector.tensor_scalar_add(g2, g2, bw2c[:, fi:fi + 1])
                    # g_sb = 16*g = (g2/64) * sil (16x keeps e4m3 < 240)
                    nc.vector.scalar_tensor_tensor(
                        out=g_sb[:, fi, :], in0=g2, scalar=1.0 / 64.0,
                        in1=sil, op0=mybir.AluOpType.mult,
                        op1=mybir.AluOpType.mult)

            # W3: single fp8 DR matmul per (token tile, output half)
            with tc.tile_pool(name="psW3", bufs=2, space="PSUM") as psW3, \
                 tc.tile_pool(name="otmp", bufs=2) as otmp:
                outr = out[:].rearrange("p (a c) -> p a c", a=4)
                for tt in range(4):
                    accT = psW3.tile([128, C], F32, tag="accT")
                    ts = tt * 128
                    for hh in range(2):
                        for fp in range(NF // 2):
                            nc.tensor.matmul(
                                accT[:, hh * 512:(hh + 1) * 512],
                                g_sb[:, 2 * fp:2 * fp + 2, ts:ts + 128],
                                w3_sb[:, 2 * fp:2 * fp + 2,
                                      hh * 512:(hh + 1) * 512],
                                start=(fp == 0), stop=(fp == NF // 2 - 1),
                                perf_mode=DR)
                    xout = otmp.tile([128, C], BF, tag="xo")
                    for hh in range(2):
                        asb = otmp.tile([128, 512], BF, tag="asb")
                        # accT = (16g)(32w3) = 512x ffn-out
                        nc.scalar.activation(
                            out=asb, in_=accT[:, hh * 512:(hh + 1) * 512],
                            func=mybir.ActivationFunctionType.Identity,
                            scale=1.0 / 512.0)
                        nc.vector.tensor_add(
                            xout[:, hh * 512:(hh + 1) * 512], asb,
                            out1[:, tt, hh * 512:(hh + 1) * 512])
                    nc.sync.dma_start(out=outr[:, tt, :], in_=xout)
    nc.compile()
    return nc


def _prep(x, Wq, Wk, Wv, Wo, W1, W2, W3, gamma, beta):
    f32 = np.float32
    scale = f32(1.0 / np.sqrt(HS))
    gcol = gamma.astype(f32)[:, None]

    def q8(a):
        return np.clip(a * S, -240.0, 240.0).astype(FP8)

    in_maps = []
    have_bw = bool(np.any(beta != 0))
    for c in range(8):
        b, r = c // 4, c % 4
        hh = slice(r * HPC, (r + 1) * HPC)
        # per-head [C, HS] blocks -> [C, 256] column groups
        qc = Wq[hh].transpose(1, 0, 2).reshape(C, 256).astype(f32) * scale
        kc = Wk[hh].transpose(1, 0, 2).reshape(C, 256).astype(f32)
        vc = Wv[hh].transpose(1, 0, 2).reshape(C, 256).astype(f32)
        wcat = np.concatenate([qc, kc, vc], axis=1)  # [C, 768]
        wq_g8 = q8(gcol * wcat)  # fp8, x32
        # gws from the quantized weights so the -mu fixup matches exactly
        gws = wq_g8.astype(f32).sum(axis=0).astype(BF16)
        # [C, 768] -> [128, NKC, 768] partition-major
        wqkv_p = np.ascontiguousarray(
            wq_g8.reshape(NKC, 128, 768).transpose(1, 0, 2)).reshape(128, -1)

        wo_sl = Wo[r * 256:(r + 1) * 256, :].astype(f32)  # [256, C]
        wo8 = q8(wo_sl).reshape(2, 128, C).transpose(1, 0, 2)
        wo_p = np.ascontiguousarray(wo8).reshape(128, -1)

        w1p = np.zeros((C, HIDP), f32)
        w1p[:, :HID] = W1
        w2p = np.zeros((C, HIDP), f32)
        w2p[:, :HID] = W2
        w3p = np.zeros((HIDP, C), f32)
        w3p[:HID, :] = W3
        # w1g/w2g[fi, p, kc, j] = w[kc*128+p, fi*128+j]
        w1g = q8(gcol * w1p).reshape(NKC, 128, NF, 128).transpose(2, 1, 0, 3)
        w2g = q8(gcol * w2p).reshape(NKC, 128, NF, 128).transpose(2, 1, 0, 3)
        # [NF, 128, 16, 128] -> [128, NF, 16, 128]
        w12g = np.concatenate(
            [w1g.reshape(NF, 128, 8, 128), w2g.reshape(NF, 128, 8, 128)],
            axis=2).transpose(1, 0, 2, 3)
        w12_p = np.ascontiguousarray(w12g).reshape(128, -1)
        # w3 [HIDP, C] -> [128, NF, C] partition-major on hid
        w38 = q8(w3p).reshape(NF, 128, C).transpose(1, 0, 2)
        w3_p = np.ascontiguousarray(w38).reshape(128, -1)

        xb = x[b].astype(f32)
        # x^T fp8 x32: [C, T] -> [128, NKC, T]
        x8 = q8(xb.T).reshape(NKC, 128, T).transpose(1, 0, 2)
        x8_p = np.ascontiguousarray(x8).reshape(128, -1)
        # xres rows: token J*512 + r*128 + p for chunk J
        xres_rows = np.stack(
            [xb[J * 512 + r * 128: J * 512 + (r + 1) * 128] for J in range(4)],
            axis=1)  # [128, 4, C]
        xres_p = np.ascontiguousarray(xres_rows.astype(BF16)).reshape(128, -1)

        m = {
            "xT8": x8_p,
            "xres": xres_p,
            "wqkv": wqkv_p,
            "gws": gws,
            "wo": wo_p,
            "w12": w12_p,
            "w3": w3_p,
        }
        if have_bw:
            m["bw"] = (beta.astype(f32) @ wq_g8.astype(f32)) / S
            m["bw1"] = (beta.astype(f32) @ w1p).astype(f32)
            m["bw2"] = ((beta.astype(f32) @ w2p) * S2).astype(f32)
        in_maps.append(m)
    return in_maps, have_bw


def kernel(x, Wq, Wk, Wv, Wo, W1, W2, W3, gamma, beta, _bench=None):
    x = np.asarray(x)
    in_maps, have_bw = _prep(np.asarray(x), np.asarray(Wq), np.asarray(Wk),
                             np.asarray(Wv), np.asarray(Wo), np.asarray(W1),
                             np.asarray(W2), np.asarray(W3),
                             np.asarray(gamma), np.asarray(beta))
    key = ("k2", have_bw)
    if key not in _cache:
        _cache[key] = _build(have_bw)
    nc = _cache[key]
    kw = dict(_bench) if _bench else {}
    res = run_bass_kernel_spmd(nc, in_maps, list(range(8)), **kw)
    outf = np.empty((B, T, C), np.float32)
    for c in range(8):
        b, r = c // 4, c % 4
        o = np.asarray(res.results[c]["out"]).astype(np.float32)
        o = o.reshape(128, 4, C)
        for J in range(4):
            outf[b, J * 512 + r * 128: J * 512 + (r + 1) * 128] = o[:, J, :]
    if _bench is not None:
        kernel.last_results = res
    return outf
